# revision 22
# baseline (speedup 1.0000x reference)
"""Depthwise Conv3D (3x3x3, VALID, stride 1) on 8 Trainium2 NeuronCores.

Strategy: patch-band matmul. Contraction runs over a (h,d) patch of the
input: partition p = hi*9 + di with 13 h-rows x 9 d-planes = 117. The
stationary matrix S_{f,kw}[p, q] (q = do*11 + hop, 77 columns) carries
weight w[kd, kh, kw, f] at (hi = hop+kh, di = do+kd) — folding all nine
(kh, kd) taps into ONE matmul pass. Only the 3 kw taps need separate
passes, accumulated in PSUM via a shifted moving-operand window:

  psum[q, (hp, wo)] += sum_p S_{f,kw}[p, q] * x[di, 11*hp + hi, wo + kw, f]

vs. the 9-pass H-Toeplitz scheme this is ~2.3x less PE time (free dim 77,
3 passes vs free 110, 9 passes) and shrinks the resident stationary from
14.2 MB to 3.5 MB (the graded single-dispatch time includes its load).

110 output rows = 10 patches x 11 outputs exactly (PH=13, stride 11), so
there are no partial tiles; the only input duplication is the 2-row patch
overlap in h (130/112 = 1.16x). d never tiles (9 planes = full shard).

Measured findings baked into this config (clean 105v14 loop-diff A/Bs):
- Input slabs are PADDED to 128 partitions (zeros on 117..127). A
  117-partition HWDGE DMA loses the 16-engine descriptor spread and
  costs +13 us/iter -- the 1.6 MB of zero-padding is cheaper.
- Shipping the h-overlap once + SBUF->SBUF on-chip replication is far
  worse (+85 us/iter): the small strided rects lose DMA efficiency.
- Evacuation writes MUST be unit-stride: stage is fi-major
  [QO, FG, NP, WO] so each DVE tensor_tensor (psum f32 + stride-0 bias
  -> bf16) writes contiguous WO runs. The interleaved [.., WO, FG]
  layout cost +215 us/iter in DVE time.
- I/O bf16 end-to-end (f32 PSUM accumulate): input 18.4 MB, stationary
  3.5 MB (preamble, split per-group so group 0 starts early), output
  10.8 MB per core; measured DMA floor 82 us/iter at ~355 GB/s/core,
  full kernel 86 us/iter (PE 76 us, all overlapped under DMA).

Sharding: data-parallel over (batch, D-half) -> 8 shards; weights
replicated (hint followed).
"""

import sys

sys.path.insert(0, "/opt/trn_rl_repo")

from contextlib import ExitStack

import numpy as np

B, D, H, W, F = 4, 16, 112, 112, 64
DO, HO, WO = 14, 110, 110
N_CORES = 8
DO_C = 7  # output d-planes per core
DIN_C = 9  # input d-planes per core
PH = 13  # h rows per patch
NP = 10  # patches (stride 11; 10*11 = 110 output rows exactly)
P_IN = PH * DIN_C  # 117 contraction partitions
QO = DO_C * 11  # 77 stationary columns = psum partitions
FG = 8  # channels per group
G = F // FG  # 8 groups
HPG = [(0, 4), (4, 8), (8, 10)]  # hp slices per psum tile (<=440 cols)
UNROLL = 7  # steady-state iterations per hardware-loop body

_cached = None

# variant flags (bisect aids; defaults = shipping config)
PAD_IN = True  # pad input slab partitions 117 -> 128 for HWDGE engine spread
SWDGE_OUT = False  # drain stage via gpsimd SWDGE instead of sync HWDGE
SPLIT_MM = False  # single-free-dim moving APs (one matmul per hp)
DEDUP_IN = False  # ship h-overlap rows once; replicate on-chip (SBUF->SBUF)
XG_BUFS = 3  # input slab double/triple buffering
STAGE_BUFS = 3
DRAIN_SPLIT = 2  # drains per group (fi-major stage layout => contiguous runs)
IN_SPLIT = 1  # input DMAs per group
KW_OUTER = False  # kw-outer MM order: 3 consecutive MMs share one stationary


def _build(
    loop_n: int = 1,
    pad_in=None,
    swdge_out=None,
    split_mm=None,
    dedup_in=None,
    kw_outer=None,
    do_in=True,
    do_mm=True,
    do_evac=True,
    do_drain=True,
):
    from concourse import bacc, mybir, tile

    pad_in = PAD_IN if pad_in is None else pad_in
    swdge_out = SWDGE_OUT if swdge_out is None else swdge_out
    split_mm = SPLIT_MM if split_mm is None else split_mm
    dedup_in = DEDUP_IN if dedup_in is None else dedup_in
    kw_outer = KW_OUTER if kw_outer is None else kw_outer
    if dedup_in:
        pad_in = False
    p_dma = 128 if pad_in else P_IN

    nc = bacc.Bacc("TRN2", target_bir_lowering=False, debug=False, num_devices=N_CORES)
    f32 = mybir.dt.float32
    bf16 = mybir.dt.bfloat16

    x_ap = nc.dram_tensor("xpk", [G, p_dma, FG, NP, W], bf16, kind="ExternalInput").ap()
    s_ap = nc.dram_tensor("spk", [P_IN, 3, F, QO], bf16, kind="ExternalInput").ap()
    b_ap = nc.dram_tensor("biasbc", [128, F], f32, kind="ExternalInput").ap()
    o_ap = nc.dram_tensor("out", [G, QO, FG, NP, WO], bf16, kind="ExternalOutput").ap()

    with tile.TileContext(nc) as tc, ExitStack() as ctx:
        res_pool = ctx.enter_context(tc.tile_pool(name="res", bufs=1))
        x_pool = ctx.enter_context(tc.tile_pool(name="xg", bufs=XG_BUFS))
        stage_pool = ctx.enter_context(tc.tile_pool(name="stage", bufs=STAGE_BUFS))
        psum_pool = ctx.enter_context(tc.tile_pool(name="psum", bufs=8, space="PSUM"))

        bias_t = res_pool.tile([128, F], f32, name="bias_t")
        nc.sync.dma_start(out=bias_t[:], in_=b_ap[:])
        s_res = res_pool.tile([P_IN, 3, F, QO], bf16, name="s_res")
        # per-group slices so group 0's matmuls start after 1/8 of the load
        for g in range(G):
            nc.sync.dma_start(
                out=s_res[:, :, g * FG : (g + 1) * FG, :],
                in_=s_ap[:, :, g * FG : (g + 1) * FG, :],
            )

        def body():
            for g in range(G):
                xg = x_pool.tile([p_dma, FG, NP, W], bf16, name="xg", tag="xg")
                if do_in and dedup_in:
                    # hp=0 full 13 rows; hp>=1 only rows hi>=2 (hi 0,1 are the
                    # previous patch's hi 11,12) -> replicate those on-chip
                    nc.scalar.dma_start(
                        out=xg[:, :, 0:1, :], in_=x_ap[g][:, :, 0:1, :]
                    )
                    nc.scalar.dma_start(
                        out=xg[18:P_IN, :, 1:NP, :], in_=x_ap[g][18:P_IN, :, 1:NP, :]
                    )
                    nc.sync.dma_start(
                        out=xg[0:18, :, 1:NP, :], in_=xg[99:P_IN, :, 0 : NP - 1, :]
                    )
                elif do_in:
                    fsl = FG // IN_SPLIT
                    for i in range(IN_SPLIT):
                        nc.scalar.dma_start(
                            out=xg[:, i * fsl : (i + 1) * fsl],
                            in_=x_ap[g][:, i * fsl : (i + 1) * fsl],
                        )
                elif do_mm:
                    nc.vector.memset(xg[:, 0, 0, 0:2], 0.0)
                stage = None
                if do_evac or do_drain:
                    stage = stage_pool.tile(
                        [QO, FG, NP, WO], bf16, name="stage", tag="stage"
                    )
                    if not (do_mm and do_evac):
                        # stripped configs: give the drain a writer
                        nc.vector.memset(stage[:, 0, 0, :], 0.0)
                for fi in range(FG) if do_mm else []:
                    f = g * FG + fi
                    if kw_outer:
                        psums = [
                            psum_pool.tile([QO, 512], f32, name="psum", tag="ps")
                            for _ in HPG
                        ]
                        for kw in range(3):
                            for j, (a, b2) in enumerate(HPG):
                                nc.tensor.matmul(
                                    psums[j][:, 0 : (b2 - a) * WO],
                                    lhsT=s_res[:, kw, f, :],
                                    rhs=xg[0:P_IN, fi, a:b2, kw : kw + WO],
                                    start=(kw == 0),
                                    stop=(kw == 2),
                                )
                        if do_evac:
                            for j, (a, b2) in enumerate(HPG):
                                hpn = b2 - a
                                ev_in = psums[j][:, 0 : hpn * WO].rearrange(
                                    "q (hp wo) -> q hp wo", hp=hpn
                                )
                                ev_b = (
                                    bias_t[0:QO, f : f + 1]
                                    .unsqueeze(2)
                                    .broadcast_to([QO, hpn, WO])
                                )
                                nc.vector.tensor_tensor(
                                    stage[:, fi, a:b2, :],
                                    ev_in,
                                    ev_b,
                                    mybir.AluOpType.add,
                                )
                        continue
                    for a, b2 in HPG:
                        hpn = b2 - a
                        psum = psum_pool.tile([QO, 512], f32, name="psum", tag="ps")
                        for kw in range(3):
                            if split_mm:
                                for hpl in range(hpn):
                                    nc.tensor.matmul(
                                        psum[:, hpl * WO : (hpl + 1) * WO],
                                        lhsT=s_res[:, kw, f, :],
                                        rhs=xg[0:P_IN, fi, a + hpl, kw : kw + WO],
                                        start=(kw == 0),
                                        stop=(kw == 2),
                                    )
                            else:
                                nc.tensor.matmul(
                                    psum[:, 0 : hpn * WO],
                                    lhsT=s_res[:, kw, f, :],
                                    rhs=xg[0:P_IN, fi, a:b2, kw : kw + WO],
                                    start=(kw == 0),
                                    stop=(kw == 2),
                                )
                        if not do_evac:
                            continue
                        ev_in = psum[:, 0 : hpn * WO].rearrange(
                            "q (hp wo) -> q hp wo", hp=hpn
                        )
                        ev_b = (
                            bias_t[0:QO, f : f + 1]
                            .unsqueeze(2)
                            .broadcast_to([QO, hpn, WO])
                        )
                        nc.vector.tensor_tensor(
                            stage[:, fi, a:b2, :], ev_in, ev_b, mybir.AluOpType.add
                        )
                if not do_drain:
                    continue
                dsl = FG // DRAIN_SPLIT
                for i in range(DRAIN_SPLIT):
                    eng = nc.gpsimd if swdge_out else nc.sync
                    eng.dma_start(
                        out=o_ap[g][:, i * dsl : (i + 1) * dsl],
                        in_=stage[:, i * dsl : (i + 1) * dsl],
                    )

        n_loop = loop_n // UNROLL
        pre = loop_n - n_loop * UNROLL
        if n_loop == 1:
            pre, n_loop = loop_n, 0
        for _ in range(pre):
            body()
        if n_loop >= 2:
            with tc.For_i(0, n_loop):
                for _ in range(UNROLL):
                    body()

    nc.compile()
    return nc


def _pack_s(w: np.ndarray) -> np.ndarray:
    """[3,3,3,1,F] f32 -> [P_IN, 3, F, QO] bf16 patch-band stationary."""
    import ml_dtypes

    t = np.zeros((PH, DIN_C, 3, F, DO_C, 11), np.float32)
    hop = np.arange(11)[:, None]
    do = np.arange(DO_C)[None, :]
    for kd in range(3):
        for kh in range(3):
            for kw in range(3):
                t[hop + kh, do + kd, kw, :, do, hop] = w[kd, kh, kw, 0, :]
    return t.reshape(P_IN, 3, F, QO).astype(ml_dtypes.bfloat16)


def _pack_x(xs: np.ndarray) -> np.ndarray:
    """[DIN_C, H, W, F] f32 -> [G, p_dma, FG, NP, W] bf16 patch slab."""
    import ml_dtypes

    hidx = 11 * np.arange(NP)[None, :] + np.arange(PH)[:, None]  # [PH, NP]
    xw = xs[:, hidx, :, :]  # [di, hi, hp, w, f]
    xp = xw.transpose(1, 0, 4, 2, 3).reshape(P_IN, G, FG, NP, W)
    xp = np.ascontiguousarray(xp.transpose(1, 0, 2, 3, 4)).astype(ml_dtypes.bfloat16)
    if not PAD_IN:
        return xp
    out = np.zeros((G, 128, FG, NP, W), ml_dtypes.bfloat16)
    out[:, :P_IN] = xp
    return out


def _in_map(core: int, x: np.ndarray, spk: np.ndarray, bias_bc: np.ndarray) -> dict:
    bb, dh = divmod(core, 2)
    return {
        "xpk": _pack_x(x[bb, dh * DO_C : dh * DO_C + DIN_C]),
        "spk": spk,
        "biasbc": bias_bc,
    }


def kernel(x: np.ndarray, w: np.ndarray, b: np.ndarray) -> np.ndarray:
    global _cached
    if _cached is None:
        _cached = _build()
    nc = _cached

    from concourse.bass_utils import run_bass_kernel_spmd

    x = np.asarray(x, np.float32)
    spk = _pack_s(np.asarray(w, np.float32))
    bias_bc = np.tile(np.asarray(b, np.float32)[None, :], (128, 1))
    in_maps = [_in_map(core, x, spk, bias_bc) for core in range(N_CORES)]

    res = run_bass_kernel_spmd(nc, in_maps, list(range(N_CORES)))

    out = np.empty((B, DO, HO, WO, F), np.float32)
    for core in range(N_CORES):
        bb, dh = divmod(core, 2)
        r = np.asarray(res.results[core]["out"])  # [G, QO, FG, NP, WO]
        r = r.reshape(G, DO_C, 11, FG, NP, WO)
        out[bb, dh * DO_C : (dh + 1) * DO_C] = (
            r.transpose(1, 4, 2, 5, 0, 3).reshape(DO_C, HO, WO, F).astype(np.float32)
        )
    return out
